# revision 2
# baseline (speedup 1.0000x reference)
"""Trainium2 Bass kernel for DigitConvolutionalModel.

Model: x[B,784] -> reshape 28x28 -> 3x3 valid conv -> [B,676] -> FC(676,300)
       -> ReLU -> FC(300,10).

Strategy:
  * Fold the conv into FC1 on the host: feat @ w1 == x @ W1e where
    W1e[784,300] = C @ w1 (C = sparse conv scatter). Weight-only preprocessing.
  * Pure data parallel over 8 NeuronCores: batch shard of 8192 rows per core.
  * Per-core shard is passed pre-transposed (feature-major) so the contraction
    dim (784 = 7 chunks x 112) sits on SBUF partitions; the kernel computes
    transposed activations throughout (batch on the free axis):
        a1T[300,b] = relu(W1e.T @ xT + b1);  yT[10,b] = w2.T @ a1T + b2
    Biases are per-partition -> fold into scalar-engine activation bias.
  * Output yT[10,8192] per core, un-transposed/gathered on host.
"""

import sys

sys.path.insert(0, "/opt/trn_rl_repo")

import numpy as np

import concourse.bass as bass
import concourse.tile as tile
from concourse import bacc, mybir
from concourse.bass_utils import run_bass_kernel_spmd

# ---- problem constants (hardcoded per harness contract) ----
B = 65536
D = 784  # 28*28
H = 300
O = 10
IMG = 28
KH = KW = 3
OUT_HW = IMG - KH + 1  # 26

N_CORES = 8
BS = B // N_CORES  # 8192 rows per core

KCH = 7  # contraction chunks
KP = D // KCH  # 112 partitions per chunk
MCH = 3  # hidden chunks
MP = H // MCH  # 100
BT = 512  # batch tile (fp32 moving-operand max)
NBT = BS // BT  # 16

# matmul operand dtype: float32 (exact) | float32r (fast fp32 mode)
import os

MM_DT = (
    mybir.dt.float32r
    if os.environ.get("BASS_MM_DT", "f32") == "f32r"
    else mybir.dt.float32
)

_cache = {}


def _build_nc():
    nc = bacc.Bacc("TRN2", target_bir_lowering=False, debug=False, num_devices=N_CORES)
    xt_d = nc.declare_dram_parameter("xt", [KP, KCH, BS], mybir.dt.float32, isOutput=False)
    w1_d = nc.declare_dram_parameter("w1e", [KP, KCH * H], mybir.dt.float32, isOutput=False)
    b1_d = nc.declare_dram_parameter("b1r", [MP, MCH], mybir.dt.float32, isOutput=False)
    w2_d = nc.declare_dram_parameter("w2r", [MP, MCH * O], mybir.dt.float32, isOutput=False)
    b2_d = nc.declare_dram_parameter("b2r", [O, 1], mybir.dt.float32, isOutput=False)
    yt_d = nc.declare_dram_parameter("yt", [O, BS], mybir.dt.float32, isOutput=True)

    f32 = mybir.dt.float32

    with tile.TileContext(nc) as tc:
        with (
            tc.tile_pool(name="singles", bufs=1) as singles,
            tc.tile_pool(name="xp", bufs=3) as xp,
            tc.tile_pool(name="ap", bufs=3) as ap,
            tc.tile_pool(name="yp", bufs=3) as yp,
            tc.tile_pool(name="ps1", bufs=4, space="PSUM") as ps1p,
            tc.tile_pool(name="ps2", bufs=2, space="PSUM") as ps2p,
        ):
            w1sb = singles.tile([KP, KCH * H], f32)
            nc.sync.dma_start(w1sb[:], w1_d[:])
            b1sb = singles.tile([MP, MCH], f32)
            nc.sync.dma_start(b1sb[:], b1_d[:])
            w2sb = singles.tile([MP, MCH * O], f32)
            nc.sync.dma_start(w2sb[:], w2_d[:])
            b2sb = singles.tile([O, 1], f32)
            nc.sync.dma_start(b2sb[:], b2_d[:])

            w1mm = w1sb.bitcast(MM_DT) if MM_DT != f32 else w1sb
            w2mm = w2sb.bitcast(MM_DT) if MM_DT != f32 else w2sb

            for bt in range(NBT):
                xt = xp.tile([KP, KCH, BT], f32)
                nc.sync.dma_start(xt[:], xt_d[:, :, bt * BT : (bt + 1) * BT])
                xmm = xt.bitcast(MM_DT) if MM_DT != f32 else xt

                a1 = ap.tile([MP, MCH, BT], f32)
                for j in range(MCH):
                    ps = ps1p.tile([MP, BT], f32)
                    for k in range(KCH):
                        nc.tensor.matmul(
                            ps[:],
                            w1mm[:, k * H + j * MP : k * H + (j + 1) * MP],
                            xmm[:, k, :],
                            start=(k == 0),
                            stop=(k == KCH - 1),
                        )
                    nc.scalar.activation(
                        a1[:, j, :],
                        ps[:],
                        mybir.ActivationFunctionType.Relu,
                        bias=b1sb[:, j : j + 1],
                    )

                a1mm = a1.bitcast(MM_DT) if MM_DT != f32 else a1
                ps2 = ps2p.tile([O, BT], f32)
                for j in range(MCH):
                    nc.tensor.matmul(
                        ps2[:],
                        w2mm[:, j * O : (j + 1) * O],
                        a1mm[:, j, :],
                        start=(j == 0),
                        stop=(j == MCH - 1),
                    )
                yt = yp.tile([O, BT], f32)
                nc.vector.tensor_scalar_add(yt[:], ps2[:], b2sb[:, 0:1])
                nc.sync.dma_start(yt_d[:, bt * BT : (bt + 1) * BT], yt[:])

    nc.compile()
    return nc


def _host_prep_weights(conv_w, w1, b1, w2, b2):
    # Fold conv into FC1: W1e = C @ w1, computed in f64 then cast.
    w1g = w1.astype(np.float64).reshape(OUT_HW, OUT_HW, H)
    w1e = np.zeros((IMG, IMG, H), dtype=np.float64)
    cw = conv_w.astype(np.float64)
    for di in range(KH):
        for dj in range(KW):
            w1e[di : di + OUT_HW, dj : dj + OUT_HW, :] += cw[di, dj] * w1g
    w1e = w1e.reshape(D, H).astype(np.float32)

    w1e_r = np.ascontiguousarray(
        w1e.reshape(KCH, KP, H).transpose(1, 0, 2).reshape(KP, KCH * H)
    )
    b1_r = np.ascontiguousarray(b1.reshape(MCH, MP).T)
    w2_r = np.ascontiguousarray(
        w2.reshape(MCH, MP, O).transpose(1, 0, 2).reshape(MP, MCH * O)
    )
    b2_r = np.ascontiguousarray(b2.reshape(O, 1))
    return w1e_r, b1_r, w2_r, b2_r


def kernel(x, conv_w, w1, b1, w2, b2):
    x = np.asarray(x, dtype=np.float32)
    w1e_r, b1_r, w2_r, b2_r = _host_prep_weights(
        np.asarray(conv_w, np.float32),
        np.asarray(w1, np.float32),
        np.asarray(b1, np.float32),
        np.asarray(w2, np.float32),
        np.asarray(b2, np.float32),
    )

    if "nc" not in _cache:
        _cache["nc"] = _build_nc()
    nc = _cache["nc"]

    in_maps = []
    for c in range(N_CORES):
        xc = x[c * BS : (c + 1) * BS]  # [BS, 784]
        # xt[p, k, b] = x[b, k*KP + p]
        xt = np.ascontiguousarray(xc.T.reshape(KCH, KP, BS).transpose(1, 0, 2))
        in_maps.append(
            {"xt": xt, "w1e": w1e_r, "b1r": b1_r, "w2r": w2_r, "b2r": b2_r}
        )

    res = run_bass_kernel_spmd(nc, in_maps, list(range(N_CORES)))

    y = np.empty((B, O), dtype=np.float32)
    for c in range(N_CORES):
        y[c * BS : (c + 1) * BS] = res.results[c]["yt"].T
    return y


# revision 3
# speedup vs baseline: 6901.1605x; 6901.1605x over previous
"""Trainium2 Bass kernel for DigitConvolutionalModel.

Model: x[B,784] -> reshape 28x28 -> 3x3 valid conv -> [B,676] -> FC(676,300)
       -> ReLU -> FC(300,10).

Strategy:
  * Fold the conv into FC1 on the host: feat @ w1 == x @ W1e where
    W1e[784,300] = C @ w1 (C = sparse conv scatter). Weight-only preprocessing.
  * Pure data parallel over 8 NeuronCores: batch shard of 8192 rows per core.
  * Per-core shard is passed pre-transposed (feature-major) so the contraction
    dim (784 = 7 chunks x 112) sits on SBUF partitions; the kernel computes
    transposed activations throughout (batch on the free axis):
        a1T[300,b] = relu(W1e.T @ xT + b1);  yT[10,b] = w2.T @ a1T + b2
    Biases are per-partition -> fold into scalar-engine activation bias.
  * Output yT[10,8192] per core, un-transposed/gathered on host.
"""

import sys

sys.path.insert(0, "/opt/trn_rl_repo")

import numpy as np

import concourse.bass as bass
import concourse.tile as tile
from concourse import bacc, mybir
from concourse.bass_utils import run_bass_kernel_spmd

# ---- problem constants (hardcoded per harness contract) ----
B = 65536
D = 784  # 28*28
H = 300
O = 10
IMG = 28
KH = KW = 3
OUT_HW = IMG - KH + 1  # 26

N_CORES = 8
BS = B // N_CORES  # 8192 rows per core

KCH = 7  # contraction chunks
KP = D // KCH  # 112 partitions per chunk
MCH = 3  # hidden chunks
MP = H // MCH  # 100
BT = 512  # batch tile (fp32 moving-operand max)
NBT = BS // BT  # 16

# matmul operand dtype: float32 (exact) | float32r (fast fp32 mode)
import os

MM_DT = (
    mybir.dt.float32r
    if os.environ.get("BASS_MM_DT", "f32") == "f32r"
    else mybir.dt.float32
)

_cache = {}


def _build_nc():
    f32 = mybir.dt.float32
    mdt = MM_DT

    nc = bacc.Bacc("TRN2", target_bir_lowering=False, debug=False, num_devices=N_CORES)
    xt_d = nc.declare_dram_parameter("xt", [KP, KCH, BS], mdt, isOutput=False)
    w1_d = nc.declare_dram_parameter("w1e", [KP, KCH * H], mdt, isOutput=False)
    b1_d = nc.declare_dram_parameter("b1r", [MP, MCH], f32, isOutput=False)
    w2_d = nc.declare_dram_parameter("w2r", [MP, MCH * O], mdt, isOutput=False)
    b2_d = nc.declare_dram_parameter("b2r", [O, 1], f32, isOutput=False)
    yt_d = nc.declare_dram_parameter("yt", [O, BS], f32, isOutput=True)

    with tile.TileContext(nc) as tc:
        with (
            tc.tile_pool(name="singles", bufs=1) as singles,
            tc.tile_pool(name="xp", bufs=3) as xp,
            tc.tile_pool(name="ap", bufs=3) as ap,
            tc.tile_pool(name="yp", bufs=3) as yp,
            tc.tile_pool(name="ps1", bufs=4, space="PSUM") as ps1p,
            tc.tile_pool(name="ps2", bufs=2, space="PSUM") as ps2p,
        ):
            w1sb = singles.tile([KP, KCH * H], mdt)
            nc.sync.dma_start(w1sb[:], w1_d[:])
            b1sb = singles.tile([MP, MCH], f32)
            nc.sync.dma_start(b1sb[:], b1_d[:])
            w2sb = singles.tile([MP, MCH * O], mdt)
            nc.sync.dma_start(w2sb[:], w2_d[:])
            b2sb = singles.tile([O, 1], f32)
            nc.sync.dma_start(b2sb[:], b2_d[:])

            for bt in range(NBT):
                xt = xp.tile([KP, KCH, BT], mdt)
                nc.sync.dma_start(xt[:], xt_d[:, :, bt * BT : (bt + 1) * BT])

                a1 = ap.tile([MP, MCH, BT], mdt)
                for j in range(MCH):
                    ps = ps1p.tile([MP, BT], f32)
                    for k in range(KCH):
                        nc.tensor.matmul(
                            ps[:],
                            w1sb[:, k * H + j * MP : k * H + (j + 1) * MP],
                            xt[:, k, :],
                            start=(k == 0),
                            stop=(k == KCH - 1),
                        )
                    nc.scalar.activation(
                        a1[:, j, :],
                        ps[:],
                        mybir.ActivationFunctionType.Relu,
                        bias=b1sb[:, j : j + 1],
                    )

                ps2 = ps2p.tile([O, BT], f32)
                for j in range(MCH):
                    nc.tensor.matmul(
                        ps2[:],
                        w2sb[:, j * O : (j + 1) * O],
                        a1[:, j, :],
                        start=(j == 0),
                        stop=(j == MCH - 1),
                    )
                yt = yp.tile([O, BT], f32)
                nc.vector.tensor_scalar_add(yt[:], ps2[:], b2sb[:, 0:1])
                nc.sync.dma_start(yt_d[:, bt * BT : (bt + 1) * BT], yt[:])

    nc.compile()
    return nc


def _host_prep_weights(conv_w, w1, b1, w2, b2):
    # Fold conv into FC1: W1e = C @ w1, computed in f64 then cast.
    w1g = w1.astype(np.float64).reshape(OUT_HW, OUT_HW, H)
    w1e = np.zeros((IMG, IMG, H), dtype=np.float64)
    cw = conv_w.astype(np.float64)
    for di in range(KH):
        for dj in range(KW):
            w1e[di : di + OUT_HW, dj : dj + OUT_HW, :] += cw[di, dj] * w1g
    w1e = w1e.reshape(D, H).astype(np.float32)

    w1e_r = np.ascontiguousarray(
        w1e.reshape(KCH, KP, H).transpose(1, 0, 2).reshape(KP, KCH * H)
    )
    b1_r = np.ascontiguousarray(b1.reshape(MCH, MP).T)
    w2_r = np.ascontiguousarray(
        w2.reshape(MCH, MP, O).transpose(1, 0, 2).reshape(MP, MCH * O)
    )
    b2_r = np.ascontiguousarray(b2.reshape(O, 1))
    return w1e_r, b1_r, w2_r, b2_r


def kernel(x, conv_w, w1, b1, w2, b2):
    x = np.asarray(x, dtype=np.float32)
    w1e_r, b1_r, w2_r, b2_r = _host_prep_weights(
        np.asarray(conv_w, np.float32),
        np.asarray(w1, np.float32),
        np.asarray(b1, np.float32),
        np.asarray(w2, np.float32),
        np.asarray(b2, np.float32),
    )

    if "nc" not in _cache:
        _cache["nc"] = _build_nc()
    nc = _cache["nc"]

    in_maps = []
    for c in range(N_CORES):
        xc = x[c * BS : (c + 1) * BS]  # [BS, 784]
        # xt[p, k, b] = x[b, k*KP + p]
        xt = np.ascontiguousarray(xc.T.reshape(KCH, KP, BS).transpose(1, 0, 2))
        in_maps.append(
            {"xt": xt, "w1e": w1e_r, "b1r": b1_r, "w2r": w2_r, "b2r": b2_r}
        )

    res = run_bass_kernel_spmd(nc, in_maps, list(range(N_CORES)))

    y = np.empty((B, O), dtype=np.float32)
    for c in range(N_CORES):
        y[c * BS : (c + 1) * BS] = res.results[c]["yt"].T
    return y


# revision 12
# speedup vs baseline: 46225.0383x; 6.6982x over previous
"""Trainium2 Bass kernel for DigitConvolutionalModel.

Model: x[B,784] -> reshape 28x28 -> 3x3 valid conv -> [B,676] -> FC(676,300)
       -> ReLU -> FC(300,10).

Strategy:
  * Fold the conv into FC1 on the host: feat @ w1 == x @ W1e where
    W1e[784,300] = C @ w1 (C = sparse conv scatter). Weight-only preprocessing.
  * Pure data parallel over 8 NeuronCores: batch shard of 8192 rows per core.
  * Per-core shard is passed pre-transposed (feature-major) so the contraction
    dim (784 = 7 chunks x 112) sits on SBUF partitions; the kernel computes
    transposed activations throughout (batch on the free axis):
        a1T[300,b] = relu(W1e.T @ xT + b1);  yT[10,b] = w2.T @ a1T + b2
    Biases are per-partition -> fold into scalar-engine activation bias.
  * float32r matmul operands: full-rate PE streaming with fp32 PSUM
    accumulation (~2e-4 rel err vs the fp32 reference).
  * Output yT[10,8192] per core, un-transposed/gathered on host.
"""

import os
import sys

sys.path.insert(0, "/opt/trn_rl_repo")

import numpy as np

import concourse.bass as bass
import concourse.tile as tile
from concourse import bacc, mybir
from concourse.bass_utils import run_bass_kernel_spmd

# ---- problem constants (hardcoded per harness contract) ----
B = 65536
D = 784  # 28*28
H = 300
O = 10
IMG = 28
KH = KW = 3
OUT_HW = IMG - KH + 1  # 26

N_CORES = 8
BS = B // N_CORES  # 8192 rows per core

KCH = 7  # contraction chunks
KP = D // KCH  # 112 partitions per chunk
BT = 512  # batch tile (fp32 moving-operand max)
NBT = BS // BT  # 16
MPAD = 128  # padded partition count for hidden-chunk tensors

# matmul operand dtype: float32 (exact) | float32r (fast fp32 mode)
MM_DT = (
    mybir.dt.float32
    if os.environ.get("BASS_MM_DT", "f32r") == "f32"
    else mybir.dt.float32r
)

# hidden-dim chunking (sum must be H)
if os.environ.get("M_UNEVEN", "0") == "1":
    M_CHUNKS = [128, 128, 44]
else:
    M_CHUNKS = [100, 100, 100]
M_OFFS = [sum(M_CHUNKS[:i]) for i in range(len(M_CHUNKS))]
MCH = len(M_CHUNKS)

# tunables (env-overridable for experiments)
XP_BUFS = int(os.environ.get("XP_BUFS", "3"))
AP_BUFS = int(os.environ.get("AP_BUFS", "3"))
PS1_BUFS = int(os.environ.get("PS1_BUFS", "4"))
PS2_BUFS = int(os.environ.get("PS2_BUFS", "2"))
X_DMA_SPLIT = int(os.environ.get("X_DMA_SPLIT", "7"))  # k-chunk granularity of x loads
REPS = int(os.environ.get("KERNEL_REPS", "1"))  # timing only: repeat body in-module

_cache = {}


def _build_nc():
    f32 = mybir.dt.float32
    mdt = MM_DT

    nc = bacc.Bacc("TRN2", target_bir_lowering=False, debug=False, num_devices=N_CORES)
    xt_d = nc.declare_dram_parameter("xt", [KP, KCH, BS], mdt, isOutput=False)
    w1_d = nc.declare_dram_parameter("w1e", [KP, KCH * H], mdt, isOutput=False)
    b1_d = nc.declare_dram_parameter("b1r", [MPAD, MCH], f32, isOutput=False)
    w2_d = nc.declare_dram_parameter("w2r", [MPAD, MCH * O], mdt, isOutput=False)
    b2_d = nc.declare_dram_parameter("b2r", [O, 1], f32, isOutput=False)
    yt_d = nc.declare_dram_parameter("yt", [O, BS], f32, isOutput=True)

    with tile.TileContext(nc) as tc:
        with (
            tc.tile_pool(name="singles", bufs=1) as singles,
            tc.tile_pool(name="xp", bufs=XP_BUFS) as xp,
            tc.tile_pool(name="ap", bufs=AP_BUFS) as ap,
            tc.tile_pool(name="yp", bufs=3) as yp,
            tc.tile_pool(name="ps1", bufs=PS1_BUFS, space="PSUM") as ps1p,
            tc.tile_pool(name="ps2", bufs=PS2_BUFS, space="PSUM") as ps2p,
        ):
            w1sb = singles.tile([KP, KCH * H], mdt)
            nc.sync.dma_start(w1sb[:], w1_d[:])
            b1sb = singles.tile([MPAD, MCH], f32)
            nc.sync.dma_start(b1sb[:], b1_d[:])
            w2sb = singles.tile([MPAD, MCH * O], mdt)
            nc.sync.dma_start(w2sb[:], w2_d[:])
            b2sb = singles.tile([O, 1], f32)
            nc.sync.dma_start(b2sb[:], b2_d[:])

            for bt in [i for _ in range(REPS) for i in range(NBT)]:
                xt = xp.tile([KP, KCH, BT], mdt)
                if X_DMA_SPLIT <= 1:
                    nc.sync.dma_start(xt[:], xt_d[:, :, bt * BT : (bt + 1) * BT])
                else:
                    step = (KCH + X_DMA_SPLIT - 1) // X_DMA_SPLIT
                    for s in range(0, KCH, step):
                        e = min(s + step, KCH)
                        nc.sync.dma_start(
                            xt[:, s:e, :],
                            xt_d[:, s:e, bt * BT : (bt + 1) * BT],
                        )

                a1 = ap.tile([MPAD, MCH, BT], mdt)
                for j in range(MCH):
                    mlen, moff = M_CHUNKS[j], M_OFFS[j]
                    ps = ps1p.tile([MPAD, BT], f32)
                    for k in range(KCH):
                        nc.tensor.matmul(
                            ps[0:mlen, :],
                            w1sb[:, k * H + moff : k * H + moff + mlen],
                            xt[:, k, :],
                            start=(k == 0),
                            stop=(k == KCH - 1),
                        )
                    nc.scalar.activation(
                        a1[0:mlen, j, :],
                        ps[0:mlen, :],
                        mybir.ActivationFunctionType.Relu,
                        bias=b1sb[0:mlen, j : j + 1],
                    )

                ps2 = ps2p.tile([O, BT], f32)
                for j in range(MCH):
                    mlen = M_CHUNKS[j]
                    nc.tensor.matmul(
                        ps2[:],
                        w2sb[0:mlen, j * O : (j + 1) * O],
                        a1[0:mlen, j, :],
                        start=(j == 0),
                        stop=(j == MCH - 1),
                    )
                yt = yp.tile([O, BT], f32)
                nc.vector.tensor_scalar_add(yt[:], ps2[:], b2sb[:, 0:1])
                nc.sync.dma_start(yt_d[:, bt * BT : (bt + 1) * BT], yt[:])

    nc.compile()
    return nc


def _host_prep_weights(conv_w, w1, b1, w2, b2):
    # Fold conv into FC1: W1e = C @ w1, computed in f64 then cast.
    w1g = w1.astype(np.float64).reshape(OUT_HW, OUT_HW, H)
    w1e = np.zeros((IMG, IMG, H), dtype=np.float64)
    cw = conv_w.astype(np.float64)
    for di in range(KH):
        for dj in range(KW):
            w1e[di : di + OUT_HW, dj : dj + OUT_HW, :] += cw[di, dj] * w1g
    w1e = w1e.reshape(D, H).astype(np.float32)

    w1e_r = np.ascontiguousarray(
        w1e.reshape(KCH, KP, H).transpose(1, 0, 2).reshape(KP, KCH * H)
    )
    b1f = b1.reshape(H)
    b1_r = np.zeros((MPAD, MCH), np.float32)
    w2_r = np.zeros((MPAD, MCH * O), np.float32)
    for j in range(MCH):
        mlen, moff = M_CHUNKS[j], M_OFFS[j]
        b1_r[0:mlen, j] = b1f[moff : moff + mlen]
        w2_r[0:mlen, j * O : (j + 1) * O] = w2[moff : moff + mlen, :]
    b2_r = np.ascontiguousarray(b2.reshape(O, 1))
    return w1e_r, b1_r, w2_r, b2_r


def kernel(x, conv_w, w1, b1, w2, b2):
    x = np.asarray(x, dtype=np.float32)
    w1e_r, b1_r, w2_r, b2_r = _host_prep_weights(
        np.asarray(conv_w, np.float32),
        np.asarray(w1, np.float32),
        np.asarray(b1, np.float32),
        np.asarray(w2, np.float32),
        np.asarray(b2, np.float32),
    )

    if "nc" not in _cache:
        _cache["nc"] = _build_nc()
    nc = _cache["nc"]

    in_maps = []
    for c in range(N_CORES):
        xc = x[c * BS : (c + 1) * BS]  # [BS, 784]
        # xt[p, k, b] = x[b, k*KP + p]
        xt = np.ascontiguousarray(xc.T.reshape(KCH, KP, BS).transpose(1, 0, 2))
        in_maps.append(
            {"xt": xt, "w1e": w1e_r, "b1r": b1_r, "w2r": w2_r, "b2r": b2_r}
        )

    res = run_bass_kernel_spmd(nc, in_maps, list(range(N_CORES)))

    y = np.empty((B, O), dtype=np.float32)
    for c in range(N_CORES):
        y[c * BS : (c + 1) * BS] = res.results[c]["yt"].T
    return y
